# revision 14
# baseline (speedup 1.0000x reference)
"""Trainium2 Bass kernel for nn_Model_51788715655811 (GCN actor-critic).

Strategy (8 NeuronCores, SPMD):
  - Nodes padded 5000 -> 5120 and sharded 640/core (node-parallel).
  - Dense normalized adjacency A_hat (built host-side from edge_index) is
    used for message passing: each core computes its 640 destination rows.
  - Layer 1: Z = x @ W1 computed per-core on local nodes, AllGather to all
    cores, then U1 = A_hat @ Z_full (+b1, tanh) -> h1 local, AllGather.
  - Layer 2: transposed aggregation U2^T = H1_full^T-contraction against
    A_hat columns, then h2^T = tanh(W2^T @ U2^T + b2) written directly in
    obs^T layout [(node,d), batch].
  - Big MLP first layers: row-sharded obs^T @ [Wa1|Wb1|Wc1] with K-local
    rows, AllReduce of [16, 320] partials.
  - MLP tails + Beta distribution math (softplus, gammaln/digamma via
    degree-6 polynomials) computed feature-major on every core; per-node
    outputs column-sharded by the Wa3/Wb3 slices.
  All matmuls run as float32r (full-rate fp32 with ~11-bit mantissa).
"""

from contextlib import ExitStack

import numpy as np

B, N, F_IN, D = 16, 5000, 32, 64
NPAD = 5120
NCORES = 8
NL = NPAD // NCORES          # 640 nodes per core
MCH = NL // 128              # 5 dst chunks per core
KT = NPAD // 128             # 40 contraction tiles over nodes
KL = NL * D                  # 40960 local obs rows
JT = KL // 128               # 320 phase-2 k tiles
COLS = 320                   # 128 (a) + 128 (b) + 64 (c) output columns
BD = B * D                   # 1024
BF = B * F_IN                # 512

# Degree-6 polynomials (highest coeff first) in t = x - center.
GA_C = 1.7
GA = [0.007659547888699466, -0.016328089862381094, 0.03705580320373057,
      -0.10066872595256623, 0.39661653155327026, 0.2085477746134598,
      -0.09580769763509747]
GS_C = 3.4
GS = [0.00014543563100996918, -0.0006573091704290916, 0.0032314435425157013,
      -0.019263734006802362, 0.1707707162467929, 1.0695678054036544,
      1.09232805969272]
DA = [-0.026563798271092308, 0.046198723058238314, -0.07956515257144589,
      0.14820812308232684, -0.3020450896715853, 0.7932332399393418,
      0.20854787594630828]
DS = [-0.00024058445944667207, 0.0008760310726561661, -0.0032256479113546388,
      0.012925082169186164, -0.05779490157032255, 0.34154145876801184,
      1.069567836618606]

_CACHE = {}


def _build_program():
    import concourse.bass as bass
    import concourse.mybir as mybir
    from concourse import bacc
    from concourse.tile import TileContext

    f32 = mybir.dt.float32
    F32R = mybir.dt.float32r
    AF = mybir.ActivationFunctionType
    OP = mybir.AluOpType
    GROUPS = [list(range(NCORES))]

    def r(ap):
        return ap.bitcast(F32R)

    nc = bacc.Bacc("TRN2", target_bir_lowering=False, debug=False,
                   num_devices=NCORES)

    # ---- I/O ----
    xt = nc.dram_tensor("xt", [B, F_IN, NL], F32R, kind="ExternalInput")
    at = nc.dram_tensor("at", [NPAD, NL], F32R, kind="ExternalInput")
    wbig = nc.dram_tensor("wbig", [KL, COLS], F32R, kind="ExternalInput")
    w1 = nc.dram_tensor("w1", [F_IN, D], F32R, kind="ExternalInput")
    w2s = nc.dram_tensor("w2s", [128, D], F32R, kind="ExternalInput")
    wa2 = nc.dram_tensor("wa2", [128, 128], F32R, kind="ExternalInput")
    wb2 = nc.dram_tensor("wb2", [128, 128], F32R, kind="ExternalInput")
    wc2 = nc.dram_tensor("wc2", [64, 64], F32R, kind="ExternalInput")
    wa3 = nc.dram_tensor("wa3", [128, NL], F32R, kind="ExternalInput")
    wb3 = nc.dram_tensor("wb3", [128, NL], F32R, kind="ExternalInput")
    wc3 = nc.dram_tensor("wc3", [64, 1], F32R, kind="ExternalInput")
    bias1 = nc.dram_tensor("bias1", [128, BD], f32, kind="ExternalInput")
    b2col = nc.dram_tensor("b2col", [128, 1], f32, kind="ExternalInput")
    ba1 = nc.dram_tensor("ba1", [128, 1], f32, kind="ExternalInput")
    bb1 = nc.dram_tensor("bb1", [128, 1], f32, kind="ExternalInput")
    bc1 = nc.dram_tensor("bc1", [64, 1], f32, kind="ExternalInput")
    ba2 = nc.dram_tensor("ba2", [128, 1], f32, kind="ExternalInput")
    bb2 = nc.dram_tensor("bb2", [128, 1], f32, kind="ExternalInput")
    bc2 = nc.dram_tensor("bc2", [64, 1], f32, kind="ExternalInput")
    ba3 = nc.dram_tensor("ba3", [128, MCH], f32, kind="ExternalInput")
    bb3 = nc.dram_tensor("bb3", [128, MCH], f32, kind="ExternalInput")
    bc3 = nc.dram_tensor("bc3", [1, 1], f32, kind="ExternalInput")
    eye16 = nc.dram_tensor("eye16", [16, 16], f32, kind="ExternalInput")

    z_loc = nc.dram_tensor("z_loc", [NL, BD], F32R)
    z_full = nc.dram_tensor("z_full", [NPAD, BD], F32R, addr_space="Shared")
    h1_loc = nc.dram_tensor("h1_loc", [NL, BD], F32R)
    h1_full = nc.dram_tensor("h1_full", [NPAD, BD], F32R, addr_space="Shared")
    ar_in = nc.dram_tensor("ar_in", [16, COLS], f32)
    ar_out = nc.dram_tensor("ar_out", [16, COLS], f32, addr_space="Shared")

    actT = nc.dram_tensor("actT", [NL, 16], f32, kind="ExternalOutput")
    logpT = nc.dram_tensor("logpT", [NL, 16], f32, kind="ExternalOutput")
    entT = nc.dram_tensor("entT", [NL, 16], f32, kind="ExternalOutput")
    valT = nc.dram_tensor("valT", [1, 16], f32, kind="ExternalOutput")

    with TileContext(nc) as tc:
        with (
            tc.tile_pool(name="pres", bufs=1) as pres,
            tc.tile_pool(name="pbig", bufs=1) as pbig,
        ):
            # resident small tensors
            w1_sb = pres.tile([F_IN, D], F32R)
            nc.sync.dma_start(out=w1_sb[:], in_=w1[:])
            w2s_sb = pres.tile([128, D], F32R)
            nc.sync.dma_start(out=w2s_sb[:], in_=w2s[:])
            b2_sb = pres.tile([128, 1], f32)
            nc.sync.dma_start(out=b2_sb[:], in_=b2col[:])
            eye_sb = pres.tile([16, 16], f32)
            nc.sync.dma_start(out=eye_sb[:], in_=eye16[:])

            # ---------------- S1: Z = x @ W1 on local nodes ----------------
            with (
                tc.tile_pool(name="pxt", bufs=3) as pxt,
                tc.tile_pool(name="pzc", bufs=2) as pzc,
                tc.tile_pool(name="pz", bufs=2, space="PSUM") as pz,
            ):
                for ch in range(MCH):
                    zc = pzc.tile([128, BD], F32R, tag="zc")
                    for b in range(B):
                        xt_t = pxt.tile([F_IN, 128], F32R, tag="xt")
                        nc.sync.dma_start(
                            out=xt_t[:], in_=xt[b, :, ch * 128:(ch + 1) * 128])
                        zp = pz.tile([128, D], f32, tag="zp")
                        nc.tensor.matmul(zp[:], r(xt_t[:]), r(w1_sb[:]),
                                         start=True, stop=True)
                        nc.vector.tensor_copy(out=zc[:, b * D:(b + 1) * D],
                                              in_=zp[:])
                    nc.sync.dma_start(
                        out=z_loc[ch * 128:(ch + 1) * 128, :], in_=zc[:])

            # ---------------- S2: AllGather Z ----------------
            nc.gpsimd.collective_compute(
                "AllGather", mybir.AluOpType.bypass, replica_groups=GROUPS,
                ins=[z_loc[:]], outs=[z_full[:]])
            Zf = pbig.tile([128, KT, BD], F32R, tag="big")
            for k in range(KT):
                nc.sync.dma_start(out=Zf[:, k, :],
                                  in_=z_full[k * 128:(k + 1) * 128, :])

            # ---------------- S3: agg1 + bias + tanh ----------------
            with (
                tc.tile_pool(name="pbias", bufs=1) as pbias,
                tc.tile_pool(name="pat1", bufs=4) as pat1,
                tc.tile_pool(name="ph1m", bufs=2) as ph1m,
                tc.tile_pool(name="pg1", bufs=2, space="PSUM") as pg1,
            ):
                bias1_sb = pbias.tile([128, BD], f32)
                nc.sync.dma_start(out=bias1_sb[:], in_=bias1[:])
                for m in range(MCH):
                    psU = pg1.tile([128, BD], f32, tag="u")
                    for k in range(KT):
                        at_t = pat1.tile([128, 128], F32R, tag="at1")
                        nc.sync.dma_start(
                            out=at_t[:],
                            in_=at[k * 128:(k + 1) * 128,
                                   m * 128:(m + 1) * 128])
                        nc.tensor.matmul(psU[:, 0:512], r(at_t[:]),
                                         r(Zf[:, k, 0:512]),
                                         start=(k == 0), stop=(k == KT - 1))
                        nc.tensor.matmul(psU[:, 512:1024], r(at_t[:]),
                                         r(Zf[:, k, 512:1024]),
                                         start=(k == 0), stop=(k == KT - 1))
                    nc.vector.tensor_tensor(out=psU[:], in0=psU[:],
                                            in1=bias1_sb[:], op=OP.add)
                    h1m = ph1m.tile([128, BD], F32R, tag="h1m")
                    nc.scalar.activation(h1m[:], psU[:], AF.Tanh)
                    nc.sync.dma_start(
                        out=h1_loc[m * 128:(m + 1) * 128, :], in_=h1m[:])

            # ---------------- S4: AllGather h1 ----------------
            nc.gpsimd.collective_compute(
                "AllGather", mybir.AluOpType.bypass, replica_groups=GROUPS,
                ins=[h1_loc[:]], outs=[h1_full[:]])
            Hf = pbig.tile([128, KT, BD], F32R, tag="big")
            for k in range(KT):
                nc.sync.dma_start(out=Hf[:, k, :],
                                  in_=h1_full[k * 128:(k + 1) * 128, :])

            # ------- S5: transposed agg2: U2T[(b,d), n] = (A @ H1)^T -------
            # + S6: h2T = tanh(W2^T @ U2T + b2) -> obsT
            obsT = pbig.tile([128, JT, 16], F32R, tag="big")
            with (
                tc.tile_pool(name="pat2", bufs=3) as pat2,
                tc.tile_pool(name="pu2sb", bufs=1) as pu2sb,
                tc.tile_pool(name="pg2", bufs=8, space="PSUM") as pg2,
            ):
                u2t = pu2sb.tile([128, 8, NL], F32R)
                for p in range(2):
                    psU2 = [(pg2.tile([128, 320], f32, tag="u5",
                                      name=f"psU2a_{p}_{ci}"),
                             pg2.tile([128, 320], f32, tag="u5",
                                      name=f"psU2b_{p}_{ci}"))
                            for ci in range(4)]
                    for k in range(KT):
                        at2_t = pat2.tile([128, NL], F32R, tag="at2")
                        nc.sync.dma_start(out=at2_t[:],
                                          in_=at[k * 128:(k + 1) * 128, :])
                        for ci in range(4):
                            cc = p * 4 + ci
                            lhs = r(Hf[:, k, cc * 128:(cc + 1) * 128])
                            nc.tensor.matmul(psU2[ci][0][:], lhs,
                                             r(at2_t[:, 0:320]),
                                             start=(k == 0),
                                             stop=(k == KT - 1))
                            nc.tensor.matmul(psU2[ci][1][:], lhs,
                                             r(at2_t[:, 320:640]),
                                             start=(k == 0),
                                             stop=(k == KT - 1))
                    for ci in range(4):
                        nc.vector.tensor_copy(out=u2t[:, p * 4 + ci, 0:320],
                                              in_=psU2[ci][0][:])
                        nc.vector.tensor_copy(out=u2t[:, p * 4 + ci, 320:640],
                                              in_=psU2[ci][1][:])
                # S6 (after both passes; Hf no longer needed)
                # All matmuls at base partition 0 (f32r rejects nonzero
                # tile_position). The odd-j half of u2t is shifted down to
                # partitions 0-63 with a small SBUF->SBUF DMA first.
                with tc.tile_pool(name="ph2", bufs=2) as ph2:
                    for cc in range(8):
                        for j in range(2):
                            b = 2 * cc + j
                            if j == 0:
                                rhs_u = u2t[0:64, cc, :]
                            else:
                                ush = ph2.tile([64, NL], F32R, tag="ush")
                                nc.sync.dma_start(out=ush[:],
                                                  in_=u2t[64:128, cc, :])
                                rhs_u = ush[:]
                            psHa = pg2.tile([64, 320], f32, tag="u5")
                            psHb = pg2.tile([64, 320], f32, tag="u5")
                            lhs_w = r(w2s_sb[0:64, :])
                            nc.tensor.matmul(psHa[:], lhs_w,
                                             r(rhs_u[:, 0:320]),
                                             start=True, stop=True)
                            nc.tensor.matmul(psHb[:], lhs_w,
                                             r(rhs_u[:, 320:640]),
                                             start=True, stop=True)
                            h2b = ph2.tile([64, NL], F32R, tag="h2b")
                            nc.scalar.activation(h2b[:, 0:320], psHa[:],
                                                 AF.Tanh,
                                                 bias=b2_sb[0:64, :1])
                            nc.scalar.activation(h2b[:, 320:640], psHb[:],
                                                 AF.Tanh,
                                                 bias=b2_sb[0:64, :1])
                            hv = h2b[:].rearrange(
                                "p (jj two) -> p two jj", two=2)
                            nc.sync.dma_start(out=obsT[0:64, :, b],
                                              in_=hv[:, 0, :])
                            nc.sync.dma_start(out=obsT[64:128, :, b],
                                              in_=hv[:, 1, :])

            # ---------------- S7: big matmul (row-sharded) ----------------
            _es = ExitStack()
            ptail = _es.enter_context(
                tc.tile_pool(name="ptail", bufs=4, space="PSUM"))
            psY = ptail.tile([16, COLS], f32, tag="t")
            with tc.tile_pool(name="pwb", bufs=3) as pwb:
                for jj in range(JT // 4):
                    wb_t = pwb.tile([128, 4, COLS], F32R, tag="wb")
                    nc.sync.dma_start(
                        out=wb_t[:],
                        in_=wbig[jj * 512:(jj + 1) * 512, :].rearrange(
                            "(q p) n -> p q n", p=128))
                    for q in range(4):
                        j = jj * 4 + q
                        nc.tensor.matmul(psY[:], r(obsT[:, j, :]),
                                         r(wb_t[:, q, :]),
                                         start=(j == 0), stop=(j == JT - 1))
            y_sb = pres.tile([16, COLS], f32)
            nc.vector.tensor_copy(out=y_sb[:], in_=psY[:])
            nc.sync.dma_start(out=ar_in[:], in_=y_sb[:])

            # ---------------- S8: AllReduce partials ----------------
            nc.gpsimd.collective_compute(
                "AllReduce", mybir.AluOpType.add, replica_groups=GROUPS,
                ins=[ar_in[:]], outs=[ar_out[:]])
            y2_sb = pres.tile([16, COLS], f32)
            nc.sync.dma_start(out=y2_sb[:], in_=ar_out[:])

            # ---------------- S9: MLP tails (feature-major) ----------------
            with tc.tile_pool(name="pw3", bufs=1) as pw3:
                wa2_sb = pw3.tile([128, 128], F32R)
                nc.sync.dma_start(out=wa2_sb[:], in_=wa2[:])
                wb2_sb = pw3.tile([128, 128], F32R)
                nc.sync.dma_start(out=wb2_sb[:], in_=wb2[:])
                wc2_sb = pw3.tile([64, 64], F32R)
                nc.sync.dma_start(out=wc2_sb[:], in_=wc2[:])
                wa3_sb = pw3.tile([128, NL], F32R)
                nc.sync.dma_start(out=wa3_sb[:], in_=wa3[:])
                wb3_sb = pw3.tile([128, NL], F32R)
                nc.sync.dma_start(out=wb3_sb[:], in_=wb3[:])
                wc3_sb = pw3.tile([64, 1], F32R)
                nc.sync.dma_start(out=wc3_sb[:], in_=wc3[:])
                bias_sb = {}
                for name, t, p in [("ba1", ba1, 128), ("bb1", bb1, 128),
                                   ("bc1", bc1, 64), ("ba2", ba2, 128),
                                   ("bb2", bb2, 128), ("bc2", bc2, 64),
                                   ("bc3", bc3, 1)]:
                    bias_sb[name] = pw3.tile([p, 1], f32, name=f"bias_{name}")
                    nc.sync.dma_start(out=bias_sb[name][:], in_=t[:])
                ba3_sb = pw3.tile([128, MCH], f32)
                nc.sync.dma_start(out=ba3_sb[:], in_=ba3[:])
                bb3_sb = pw3.tile([128, MCH], f32)
                nc.sync.dma_start(out=bb3_sb[:], in_=bb3[:])

                # transpose y2 [16, 320] -> feature-major pieces
                yT = {}
                for name, lo, w in [("a", 0, 128), ("b", 128, 128),
                                    ("c", 256, 64)]:
                    psT = ptail.tile([128, 16], f32, tag="t")
                    nc.tensor.transpose(psT[0:w, :], y2_sb[:, lo:lo + w],
                                        eye_sb[:])
                    yT[name] = pw3.tile([128, 16], f32, tag=f"yT{name}", name=f"yT_{name}")
                    nc.vector.tensor_copy(out=yT[name][0:w, :],
                                          in_=psT[0:w, :])

                def actor_branch(yTx, w2_sb, w3_sb, b1n, b2n, b3_sb):
                    a1 = pw3.tile([128, 16], F32R)
                    nc.scalar.activation(a1[:], yTx[:], AF.Tanh,
                                         bias=bias_sb[b1n][:, :1])
                    ps2 = ptail.tile([128, 16], f32, tag="t")
                    nc.tensor.matmul(ps2[:], r(w2_sb[:]), r(a1[:]),
                                     start=True, stop=True)
                    a2 = pw3.tile([128, 16], F32R)
                    nc.scalar.activation(a2[:], ps2[:], AF.Tanh,
                                         bias=bias_sb[b2n][:, :1])
                    zT = pw3.tile([128, MCH, 16], f32)
                    for m in range(MCH):
                        ps3 = ptail.tile([128, 16], f32, tag="t")
                        nc.tensor.matmul(
                            ps3[:], r(w3_sb[:, m * 128:(m + 1) * 128]),
                            r(a2[:]), start=True, stop=True)
                        nc.scalar.activation(zT[:, m, :], ps3[:],
                                             AF.Identity,
                                             bias=b3_sb[:, m:m + 1])
                    return zT

                zaT = actor_branch(yT["a"], wa2_sb, wa3_sb, "ba1", "ba2",
                                   ba3_sb)
                zbT = actor_branch(yT["b"], wb2_sb, wb3_sb, "bb1", "bb2",
                                   bb3_sb)

                # critic
                c1 = pw3.tile([64, 16], F32R)
                nc.scalar.activation(c1[:], yT["c"][0:64, :], AF.Tanh,
                                     bias=bias_sb["bc1"][:, :1])
                psc = ptail.tile([64, 16], f32, tag="t")
                nc.tensor.matmul(psc[:], r(wc2_sb[:]), r(c1[:]),
                                 start=True, stop=True)
                c2 = pw3.tile([64, 16], F32R)
                nc.scalar.activation(c2[:], psc[:], AF.Tanh,
                                     bias=bias_sb["bc2"][:, :1])
                psv = ptail.tile([1, 16], f32, tag="t")
                nc.tensor.matmul(psv[:], r(wc3_sb[:]), r(c2[:]),
                                 start=True, stop=True)
                val_sb = pw3.tile([1, 16], f32)
                nc.scalar.activation(val_sb[:], psv[:], AF.Identity,
                                     bias=bias_sb["bc3"][:, :1])
                nc.sync.dma_start(out=valT[:], in_=val_sb[:])

                # ---------------- S10: Beta distribution math ----------------
                sh = [128, MCH, 16]

                _tn = [0]

                def tile():
                    _tn[0] += 1
                    return pw3.tile(sh, f32, name=f"bm_{_tn[0]}")

                def horner(t, coeffs):
                    acc = tile()
                    nc.vector.tensor_scalar(out=acc[:], in0=t[:],
                                            scalar1=float(coeffs[0]),
                                            scalar2=float(coeffs[1]),
                                            op0=OP.mult, op1=OP.add)
                    for cf in coeffs[2:]:
                        nc.vector.tensor_tensor(out=acc[:], in0=acc[:],
                                                in1=t[:], op=OP.mult)
                        nc.vector.tensor_scalar_add(out=acc[:], in0=acc[:],
                                                    scalar1=float(cf))
                    return acc

                # softplus(z) = ln(1 + exp(z))
                spa = tile()
                nc.scalar.activation(spa[:], zaT[:], AF.Exp)
                nc.vector.tensor_scalar_add(out=spa[:], in0=spa[:],
                                            scalar1=1.0)
                nc.scalar.activation(spa[:], spa[:], AF.Ln)
                spb = tile()
                nc.scalar.activation(spb[:], zbT[:], AF.Exp)
                nc.vector.tensor_scalar_add(out=spb[:], in0=spb[:],
                                            scalar1=1.0)
                nc.scalar.activation(spb[:], spb[:], AF.Ln)
                alf = tile()
                nc.vector.tensor_scalar_add(out=alf[:], in0=spa[:],
                                            scalar1=1.0)
                bet = tile()
                nc.vector.tensor_scalar_add(out=bet[:], in0=spb[:],
                                            scalar1=1.0)
                s = tile()
                nc.vector.tensor_add(out=s[:], in0=alf[:], in1=bet[:])
                rs = tile()
                nc.vector.reciprocal(out=rs[:], in_=s[:])
                act = tile()
                nc.vector.tensor_mul(out=act[:], in0=alf[:], in1=rs[:])
                nc.sync.dma_start(
                    out=actT.rearrange("(m p) b -> p m b", p=128),
                    in_=act[:])

                la = tile()
                nc.scalar.activation(la[:], alf[:], AF.Ln)
                lb = tile()
                nc.scalar.activation(lb[:], bet[:], AF.Ln)
                ls = tile()
                nc.scalar.activation(ls[:], s[:], AF.Ln)

                ta = tile()
                nc.vector.tensor_scalar_add(out=ta[:], in0=alf[:],
                                            scalar1=-GA_C)
                tb = tile()
                nc.vector.tensor_scalar_add(out=tb[:], in0=bet[:],
                                            scalar1=-GA_C)
                ts_ = tile()
                nc.vector.tensor_scalar_add(out=ts_[:], in0=s[:],
                                            scalar1=-GS_C)

                gla = horner(ta, GA)
                glb = horner(tb, GA)
                gls = horner(ts_, GS)
                logB = tile()
                nc.vector.tensor_add(out=logB[:], in0=gla[:], in1=glb[:])
                nc.vector.tensor_sub(out=logB[:], in0=logB[:], in1=gls[:])

                # logp = spa*(la-ls) + spb*(lb-ls) - logB
                t1 = tile()
                nc.vector.tensor_sub(out=t1[:], in0=la[:], in1=ls[:])
                nc.vector.tensor_mul(out=t1[:], in0=t1[:], in1=spa[:])
                t2 = tile()
                nc.vector.tensor_sub(out=t2[:], in0=lb[:], in1=ls[:])
                nc.vector.tensor_mul(out=t2[:], in0=t2[:], in1=spb[:])
                logp = tile()
                nc.vector.tensor_add(out=logp[:], in0=t1[:], in1=t2[:])
                nc.vector.tensor_sub(out=logp[:], in0=logp[:], in1=logB[:])
                nc.sync.dma_start(
                    out=logpT.rearrange("(m p) b -> p m b", p=128),
                    in_=logp[:])

                # ent = logB - spa*dga - spb*dgb + (s-2)*dgs
                dga = horner(ta, DA)
                nc.vector.tensor_mul(out=dga[:], in0=dga[:], in1=spa[:])
                dgb = horner(tb, DA)
                nc.vector.tensor_mul(out=dgb[:], in0=dgb[:], in1=spb[:])
                dgs = horner(ts_, DS)
                sm2 = tile()
                nc.vector.tensor_scalar_add(out=sm2[:], in0=s[:],
                                            scalar1=-2.0)
                nc.vector.tensor_mul(out=dgs[:], in0=dgs[:], in1=sm2[:])
                ent = tile()
                nc.vector.tensor_sub(out=ent[:], in0=logB[:], in1=dga[:])
                nc.vector.tensor_sub(out=ent[:], in0=ent[:], in1=dgb[:])
                nc.vector.tensor_add(out=ent[:], in0=ent[:], in1=dgs[:])
                nc.sync.dma_start(
                    out=entT.rearrange("(m p) b -> p m b", p=128),
                    in_=ent[:])
            _es.close()

    nc.compile()
    return nc


def _host_prep(inputs):
    x = np.asarray(inputs["x"], np.float32)
    ei = np.asarray(inputs["edge_index"])
    g = {k: np.asarray(v, np.float32) for k, v in inputs.items()
         if k not in ("x", "edge_index")}

    src = np.concatenate([ei[0], np.arange(N, dtype=ei.dtype)])
    dst = np.concatenate([ei[1], np.arange(N, dtype=ei.dtype)])
    deg = np.zeros(N, np.float32)
    np.add.at(deg, dst, np.float32(1.0))
    dinv = (1.0 / np.sqrt(deg)).astype(np.float32)
    norm = (dinv[src] * dinv[dst]).astype(np.float64)
    AT = np.zeros((NPAD, NPAD), np.float64)
    np.add.at(AT, (src, dst), norm)      # AT[src, dst] = A_hat[dst, src]
    AT = AT.astype(np.float32)

    xt_full = np.zeros((B, F_IN, NPAD), np.float32)
    xt_full[:, :, :N] = x.transpose(0, 2, 1)

    wbig_full = np.concatenate(
        [g["Wa1"], g["Wb1"], g["Wc1"]], axis=1)      # [N*D, 320]

    def pad_cols(w, lo, hi):
        # w: [k, N] -> [k, NL] slice of node columns [lo, hi) with zero pad
        out = np.zeros((w.shape[0], NL), np.float32)
        real = max(0, min(hi, N) - lo)
        if real > 0:
            out[:, :real] = w[:, lo:lo + real]
        return out

    b1t = np.tile(g["b1"], B)                        # [1024]
    bias1 = np.repeat(b1t[None, :], 128, axis=0).astype(np.float32)
    w2s = np.concatenate([g["W2"], g["W2"]], axis=0).astype(np.float32)
    eye = np.eye(16, dtype=np.float32)

    in_maps = []
    for c in range(NCORES):
        lo = c * NL
        hi = lo + NL
        rows_lo = lo * D
        wb_c = np.zeros((KL, COLS), np.float32)
        real_rows = max(0, min(hi, N) - lo) * D
        if real_rows > 0:
            wb_c[:real_rows] = wbig_full[rows_lo:rows_lo + real_rows]
        ba3_c = np.zeros(NL, np.float32)
        bb3_c = np.zeros(NL, np.float32)
        nreal = max(0, min(hi, N) - lo)
        ba3_c[:nreal] = g["ba3"][lo:lo + nreal]
        bb3_c[:nreal] = g["bb3"][lo:lo + nreal]
        m = {
            "xt": np.ascontiguousarray(xt_full[:, :, lo:hi]),
            "at": np.ascontiguousarray(AT[:, lo:hi]),
            "wbig": wb_c,
            "w1": g["W1"],
            "w2s": w2s,
            "wa2": g["Wa2"],
            "wb2": g["Wb2"],
            "wc2": g["Wc2"],
            "wa3": pad_cols(g["Wa3"], lo, hi),
            "wb3": pad_cols(g["Wb3"], lo, hi),
            "wc3": g["Wc3"].reshape(64, 1),
            "bias1": bias1,
            "b2col": np.concatenate([g["b2"], g["b2"]]).reshape(128, 1),
            "ba1": g["ba1"].reshape(128, 1),
            "bb1": g["bb1"].reshape(128, 1),
            "bc1": g["bc1"].reshape(64, 1),
            "ba2": g["ba2"].reshape(128, 1),
            "bb2": g["bb2"].reshape(128, 1),
            "bc2": g["bc2"].reshape(64, 1),
            "ba3": np.ascontiguousarray(
                ba3_c.reshape(MCH, 128).T),
            "bb3": np.ascontiguousarray(
                bb3_c.reshape(MCH, 128).T),
            "bc3": g["bc3"].reshape(1, 1),
            "eye16": eye,
        }
        in_maps.append(m)
    return in_maps


def kernel(**inputs):
    from concourse.bass_utils import run_bass_kernel_spmd

    if "nc" not in _CACHE:
        _CACHE["nc"] = _build_program()
    nc = _CACHE["nc"]
    in_maps = _host_prep(inputs)
    res = run_bass_kernel_spmd(nc, in_maps, core_ids=list(range(NCORES)))
    return _assemble(res.results)


def _assemble(results):
    act = np.concatenate([r["actT"] for r in results], axis=0)[:N].T
    logp = np.concatenate([r["logpT"] for r in results], axis=0)[:N].T
    ent = np.concatenate([r["entT"] for r in results], axis=0)[:N].T
    value = results[0]["valT"].reshape(16, 1)
    return (np.ascontiguousarray(act.astype(np.float32)),
            np.ascontiguousarray(logp.astype(np.float32)),
            np.ascontiguousarray(ent.astype(np.float32)),
            value.astype(np.float32))
